# revision 1
# baseline (speedup 1.0000x reference)
"""GCN-style message passing kernel for Trainium2 (8 NeuronCores).

Math (see reference):
    deg    = diag(D)                      (== row sums of A by construction)
    j0(i)  = argmax_j (A[i,j] > 0)        (first neighbor; self-loops ensure >=1)
    coeff  = A * outer(1/sqrt(deg[j0]), 1/sqrt(deg))
    out    = leaky_relu((coeff @ X) @ W.T + b, 0.01)

Decomposition per core (rows sharded, 1024 rows/core):
    agg   = diag(r0) @ A_sh @ (diag(r) @ X)       r = 1/sqrt(deg), r0 = 1/sqrt(deg[j0])
    out   = leaky_relu(agg @ W.T + b)

A is 0/1 so it is exact in bf16. The big product A_sh @ Xs runs on the
TensorEngine with A^T tiles as the stationary operand (A^T obtained via
hardware DMA-transpose on load). deg[j0] is recovered on-device:
  - a side matmul with a "position" matrix W2 (w[j] = 2^(-2*(j%64)), one
    column per 64-node chunk) produces s[i,c] whose f32 EXPONENT encodes the
    first neighbor's offset within chunk c (sum of distinct 2-bit-spaced
    powers of two can never carry into the next exponent slot),
  - bit tricks + a free-dim min-reduce give first_j = 64*c* + jl*,
  - deg[first_j] is then gathered with a tiny bilinear form:
    onehot(c*)^T @ Dmat dotted with onehot(jl*), Dmat[q,r] = deg[64q+r].
"""

import numpy as np
import ml_dtypes

BF16 = ml_dtypes.bfloat16

N_NODES = 8192
F_IN = 256
F_OUT = 256
N_CORES = 8
ROWS = N_NODES // N_CORES  # rows per core

# accuracy mode: 'exact' = f32 split into two bf16 passes (err ~1e-5),
# 'fp16' = single fp16 pass (err ~5e-4), 'bf16' = single bf16 pass (~3e-3).
EXACT = 'exact'

_BUILT = {}


def _build_nc(rows, n_nodes, f_in, f_out, mode, debug=False, repeat=1, stage=99):
    exact = (mode == 'exact') or (mode is True)
    import concourse.bass as bass
    import concourse.tile as tile
    from concourse import bacc, mybir

    f32 = mybir.dt.float32
    bf = mybir.dt.float16 if mode == 'fp16' else mybir.dt.bfloat16
    i32 = mybir.dt.int32
    u32 = mybir.dt.uint32
    Alu = mybir.AluOpType

    n_jblk = n_nodes // 128     # contraction blocks
    n_iblk = rows // 128        # output row blocks per core
    C = n_nodes // 128          # 128-node chunks (s columns) == n_jblk
    NB = n_jblk
    assert C <= 128 and n_nodes % 128 == 0 and rows % 128 == 0
    assert f_in % 128 == 0 and f_out <= 512

    nc = bacc.Bacc("TRN2", target_bir_lowering=False, debug=False)
    a_sh = nc.dram_tensor("a_sh", [rows, n_nodes], bf, kind="ExternalInput")
    dvec = nc.dram_tensor("dvec", [n_nodes], f32, kind="ExternalInput")
    if exact:
        x_in = nc.dram_tensor("x_f32", [n_nodes, f_in], f32, kind="ExternalInput")
    else:
        x_in = nc.dram_tensor("x_bf", [n_nodes, f_in], bf, kind="ExternalInput")
    w_t = nc.dram_tensor("w_t", [f_in, f_out], f32, kind="ExternalInput")
    bias_row = nc.dram_tensor("bias_row", [128, f_out], f32, kind="ExternalInput")
    w2reg_d = nc.dram_tensor("w2reg", [128, n_jblk, C], bf, kind="ExternalInput")
    ident_d = nc.dram_tensor("ident", [128, 128], bf, kind="ExternalInput")
    i2c227_d = nc.dram_tensor("i2c227", [128, C], i32, kind="ExternalInput")
    iq_d = nc.dram_tensor("iota_q", [128, C], f32, kind="ExternalInput")
    ir_d = nc.dram_tensor("iota_r", [128, 128], f32, kind="ExternalInput")
    out_d = nc.dram_tensor("out_sh", [rows, f_out], f32, kind="ExternalOutput")
    if debug:
        dbg_s = nc.dram_tensor("dbg_s", [rows, C], f32, kind="ExternalOutput")
        dbg_kmin = nc.dram_tensor("dbg_kmin", [rows, 1], i32, kind="ExternalOutput")
        dbg_dj0 = nc.dram_tensor("dbg_dj0", [rows, 1], f32, kind="ExternalOutput")
        dbg_agg = nc.dram_tensor("dbg_agg", [rows, f_in], f32, kind="ExternalOutput")
        dbg_at = nc.dram_tensor("dbg_at", [128, rows], f32, kind="ExternalOutput")
        dbg_xs = nc.dram_tensor("dbg_xs", [128, f_in], f32, kind="ExternalOutput")

    nfi = f_in // 128  # fi blocks for second matmul

    with tile.TileContext(nc) as tc:
        with (
            tc.tile_pool(name="singles", bufs=1) as singles,
            tc.tile_pool(name="apool", bufs=4) as apool,
            tc.tile_pool(name="xpool", bufs=3) as xpool,
            tc.tile_pool(name="work", bufs=2) as work,
            tc.tile_pool(name="pspool", bufs=8, space="PSUM") as pspool,
        ):
            # ---- constants / prep ----
            ident = singles.tile([128, 128], bf)
            nc.gpsimd.dma_start(ident[:], ident_d[:])
            i2c227 = singles.tile([128, C], i32)
            nc.gpsimd.dma_start(i2c227[:], i2c227_d[:])
            iq = singles.tile([128, C], f32)
            nc.gpsimd.dma_start(iq[:], iq_d[:])
            ir = singles.tile([128, 128], f32)
            nc.gpsimd.dma_start(ir[:], ir_d[:])
            bias_t = singles.tile([128, f_out], f32)
            nc.gpsimd.dma_start(bias_t[:], bias_row[:])

            # degrees: r = 1/sqrt(deg) laid out [p, nb]; Dmat[q, r] = deg[64q+r]
            dvec_t = singles.tile([128, NB], f32)
            nc.gpsimd.dma_start(dvec_t[:], dvec[:].rearrange("(nb p) -> p nb", p=128))
            dmat_f = singles.tile([C, 128], f32)
            nc.gpsimd.dma_start(dmat_f[:], dvec[:].rearrange("(q r) -> q r", r=128))
            dmat_b = singles.tile([C, 128], bf)
            nc.vector.tensor_copy(dmat_b[:], dmat_f[:])

            sq_t = singles.tile([128, NB], f32)
            nc.scalar.sqrt(sq_t[:], dvec_t[:])
            r_t = singles.tile([128, NB], f32)
            nc.vector.reciprocal(r_t[:], sq_t[:])

            # W^T in bf16 hi/lo: wthi/wtlo [128, nfi, f_out]
            wt_f = singles.tile([128, nfi, f_out], f32)
            nc.gpsimd.dma_start(
                wt_f[:], w_t[:].rearrange("(nf p) fo -> p nf fo", p=128)
            )
            wthi = singles.tile([128, nfi, f_out], bf)
            nc.vector.tensor_copy(wthi[:], wt_f[:])
            wtlo = singles.tile([128, nfi, f_out], bf)
            nc.vector.tensor_sub(wtlo[:], wt_f[:], wthi[:])

            assert repeat == 1 or not debug
            for _rep in range(repeat):
                # ---- moving operand: [Xs | W2] per j-block (bf16), + lo if exact
                xsw = singles.tile([128, n_jblk, f_in + C], bf)
                for jb in range(n_jblk):
                    nc.gpsimd.dma_start(
                        xsw[:, jb, f_in:f_in + C], w2reg_d[:, jb, :]
                    )
                if exact:
                    xs_lo = singles.tile([128, n_jblk, f_in], bf)
                for jb in range(n_jblk):
                    if exact:
                        xst = xpool.tile([128, f_in], f32, tag="xst")
                        nc.sync.dma_start(xst[:], x_in[jb * 128:(jb + 1) * 128, :])
                        xsf = xpool.tile([128, f_in], f32, tag="xsf")
                        nc.vector.tensor_scalar_mul(xsf[:], xst[:], r_t[:, jb:jb + 1])
                        nc.vector.tensor_copy(xsw[:, jb, 0:f_in], xsf[:])
                        nc.vector.tensor_sub(xs_lo[:, jb, :], xsf[:], xsw[:, jb, 0:f_in])
                    else:
                        xst = xpool.tile([128, f_in], bf, tag="xst")
                        nc.gpsimd.dma_start(xst[:], x_in[jb * 128:(jb + 1) * 128, :])
                        nc.vector.tensor_scalar_mul(
                            xsw[:, jb, 0:f_in], xst[:], r_t[:, jb:jb + 1]
                        )

                if stage <= 1:
                    for ib in range(n_iblk):
                        zz = work.tile([128, f_out], f32, tag="zz")
                        nc.vector.tensor_copy(zz[:], xsw[:, ib, 0:f_out])
                        nc.sync.dma_start(out_d[ib * 128:(ib + 1) * 128, :], zz[:])
                    continue
                # ---- main accumulation: agg = A_sh @ Xs ; s = A_sh @ W2
                ps_main = [
                    pspool.tile([128, f_in + C], f32, tag="ps", name=f"ps_main{i}")
                    for i in range(n_iblk)
                ]
                for jb in range(n_jblk):
                    aslab = apool.tile([128, rows], bf, tag="aslab")
                    nc.sync.dma_start(
                        aslab[:], a_sh[:, jb * 128:(jb + 1) * 128], transpose=True
                    )
                    if debug and jb == 0:
                        a_dump = work.tile([128, rows], f32, tag="a_dump")
                        nc.vector.tensor_copy(a_dump[:], aslab[:])
                        nc.sync.dma_start(dbg_at[:], a_dump[:])
                        x_dump = work.tile([128, f_in], f32, tag="x_dump")
                        nc.vector.tensor_copy(x_dump[:], xsw[:, jb, 0:f_in])
                        nc.sync.dma_start(dbg_xs[:], x_dump[:])
                    for ib in range(n_iblk):
                        lhsT = aslab[:, ib * 128:(ib + 1) * 128]
                        nc.tensor.matmul(
                            ps_main[ib][:, 0:f_in + C],
                            lhsT,
                            xsw[:, jb, :],
                            start=(jb == 0),
                            stop=(jb == n_jblk - 1) and not exact,
                        )
                        if exact:
                            nc.tensor.matmul(
                                ps_main[ib][:, 0:f_in],
                                lhsT,
                                xs_lo[:, jb, :],
                                start=False,
                                stop=(jb == n_jblk - 1),
                            )

                if stage <= 2:
                    for ib in range(n_iblk):
                        agg_raw = work.tile([128, f_in], f32, tag="agg_raw")
                        nc.scalar.copy(agg_raw[:], ps_main[ib][:, 0:f_in])
                        nc.sync.dma_start(out_d[ib * 128:(ib + 1) * 128, :], agg_raw[:])
                    continue
                # ---- per row-block epilogue ----
                for ib in range(n_iblk):
                    # drain psum: s and unscaled agg -> SBUF (releases the bank)
                    s_sb = work.tile([128, C], f32, tag="s_sb")
                    nc.scalar.copy(s_sb[:], ps_main[ib][:, f_in:f_in + C])
                    agg_raw = work.tile([128, f_in], f32, tag="agg_raw")
                    nc.scalar.copy(agg_raw[:], ps_main[ib][:, 0:f_in])
                    if stage <= 30:
                        continue
                    e_u = work.tile([128, C], i32, tag="e_u")
                    nc.vector.tensor_scalar(
                        e_u[:], s_sb[:].bitcast(i32), 23, None,
                        op0=Alu.logical_shift_right,
                    )
                    key = work.tile([128, C], i32, tag="key")
                    nc.vector.scalar_tensor_tensor(
                        key[:], e_u[:], -1, i2c227[:], op0=Alu.mult, op1=Alu.add
                    )
                    msk = work.tile([128, C], i32, tag="msk")
                    nc.vector.tensor_scalar(
                        msk[:], e_u[:], 0, 1 << 20, op0=Alu.is_equal, op1=Alu.mult
                    )
                    key2 = work.tile([128, C], i32, tag="key2")
                    nc.vector.tensor_tensor(key2[:], key[:], msk[:], Alu.add)
                    kmin = work.tile([128, 1], i32, tag="kmin")
                    nc.vector.tensor_reduce(
                        kmin[:], key2[:], axis=mybir.AxisListType.X, op=Alu.min
                    )
                    # kmin = 256*c + jl  (c = chunk, jl = offset in chunk)
                    jl2_i = work.tile([128, 1], i32, tag="jl2_i")
                    nc.vector.tensor_scalar(
                        jl2_i[:], kmin[:], 127, None, op0=Alu.bitwise_and
                    )
                    c128_i = work.tile([128, 1], i32, tag="c128_i")
                    nc.vector.tensor_scalar(
                        c128_i[:], kmin[:], -256, None, op0=Alu.bitwise_and
                    )
                    if stage <= 31:
                        continue
                    jl2_f = work.tile([128, 1], f32, tag="jl2_f")
                    nc.vector.tensor_copy(jl2_f[:], jl2_i[:])
                    c128_f = work.tile([128, 1], f32, tag="c128_f")
                    nc.vector.tensor_copy(c128_f[:], c128_i[:])

                    if stage <= 32:
                        continue
                    # onehots; gather deg[first_j] via oq^T @ Dmat then dot with or
                    oq = work.tile([128, C], bf, tag="oq")
                    nc.vector.tensor_scalar(
                        oq[:], iq[:], c128_f[:], None, op0=Alu.is_equal
                    )
                    orf = work.tile([128, 128], f32, tag="orf")
                    nc.vector.tensor_scalar(
                        orf[:], ir[:], jl2_f[:], None, op0=Alu.is_equal
                    )
                    if stage <= 33:
                        continue
                    p_oqT = pspool.tile([C, 128], bf, tag="ps")
                    nc.tensor.transpose(p_oqT[:], oq[:], ident[:])
                    oqT = work.tile([C, 128], bf, tag="oqT")
                    nc.scalar.copy(oqT[:], p_oqT[:])
                    if stage <= 34:
                        continue
                    t1 = pspool.tile([128, 128], f32, tag="ps")
                    nc.tensor.matmul(t1[:], oqT[:], dmat_b[:], start=True, stop=True)
                    if stage <= 35:
                        continue
                    t1s = work.tile([128, 128], f32, tag="t1s")
                    nc.scalar.copy(t1s[:], t1[:])
                    ttr_scr = work.tile([128, 128], f32, tag="ttr_scr")
                    nc.vector.tensor_tensor(ttr_scr[:], t1s[:], orf[:], Alu.mult)
                    dj0 = work.tile([128, 1], f32, tag="dj0")
                    nc.vector.reduce_sum(
                        dj0[:], ttr_scr[:], axis=mybir.AxisListType.X
                    )
                    if debug:
                        nc.sync.dma_start(dbg_s[ib * 128:(ib + 1) * 128, :], s_sb[:])
                        nc.sync.dma_start(dbg_kmin[ib * 128:(ib + 1) * 128, :], kmin[:])
                        nc.sync.dma_start(dbg_dj0[ib * 128:(ib + 1) * 128, :], dj0[:])
                    if stage <= 3:
                        nc.sync.dma_start(
                            out_d[ib * 128:(ib + 1) * 128, 0:C], s_sb[:]
                        )
                        continue
                    sq0 = work.tile([128, 1], f32, tag="sq0")
                    nc.scalar.sqrt(sq0[:], dj0[:])
                    r0 = work.tile([128, 1], f32, tag="r0")
                    nc.vector.reciprocal(r0[:], sq0[:])

                    # agg scaled by r0, cast, transpose for the W matmul
                    if exact:
                        agg_f = work.tile([128, f_in], f32, tag="agg_f")
                        nc.vector.tensor_scalar_mul(agg_f[:], agg_raw[:], r0[:])
                        agg_b = work.tile([128, f_in], bf, tag="agg_b")
                        nc.vector.tensor_copy(agg_b[:], agg_f[:])
                        agg_l = work.tile([128, f_in], bf, tag="agg_l")
                        nc.vector.tensor_sub(agg_l[:], agg_f[:], agg_b[:])
                    else:
                        agg_b = work.tile([128, f_in], bf, tag="agg_b")
                        nc.vector.tensor_scalar_mul(agg_b[:], agg_raw[:], r0[:])
                    if debug:
                        agg_dump = work.tile([128, f_in], f32, tag="agg_dump")
                        nc.vector.tensor_copy(agg_dump[:], agg_b[:])
                        nc.sync.dma_start(dbg_agg[ib * 128:(ib + 1) * 128, :], agg_dump[:])

                    aggTs = []
                    for h in range(nfi):
                        p_aT = pspool.tile([128, 128], bf, tag="ps")
                        nc.tensor.transpose(
                            p_aT[:], agg_b[:, h * 128:(h + 1) * 128], ident[:]
                        )
                        aT = work.tile([128, 128], bf, tag=f"aT{h}")
                        nc.scalar.copy(aT[:], p_aT[:])
                        aggTs.append(aT)
                    if exact:
                        aggTls = []
                        for h in range(nfi):
                            p_aT = pspool.tile([128, 128], bf, tag="ps")
                            nc.tensor.transpose(
                                p_aT[:], agg_l[:, h * 128:(h + 1) * 128], ident[:]
                            )
                            aTl = work.tile([128, 128], bf, tag=f"aTl{h}")
                            nc.scalar.copy(aTl[:], p_aT[:])
                            aggTls.append(aTl)

                    ps2 = pspool.tile([128, f_out], f32, tag="ps")
                    prods = []
                    for h in range(nfi):
                        prods.append((aggTs[h], wthi[:, h, :]))
                        prods.append((aggTs[h], wtlo[:, h, :]))
                        if exact:
                            prods.append((aggTls[h], wthi[:, h, :]))
                    for pi, (lhs, rhs) in enumerate(prods):
                        nc.tensor.matmul(
                            ps2[:], lhs[:], rhs,
                            start=(pi == 0), stop=(pi == len(prods) - 1),
                        )

                    z = work.tile([128, f_out], f32, tag="z")
                    nc.vector.tensor_add(z[:], ps2[:], bias_t[:])
                    out_t = work.tile([128, f_out], f32, tag="out_t")
                    nc.vector.scalar_tensor_tensor(
                        out_t[:], z[:], 0.01, z[:], op0=Alu.mult, op1=Alu.max
                    )
                    nc.sync.dma_start(
                        out_d[ib * 128:(ib + 1) * 128, :], out_t[:]
                    )

    nc.finalize()
    return nc


def _get_nc(rows, n_nodes, f_in, f_out, mode, debug=False, repeat=1, stage=99):
    key = (rows, n_nodes, f_in, f_out, mode, debug, repeat, stage)
    if key not in _BUILT:
        _BUILT[key] = _build_nc(*key)
    return _BUILT[key]


def host_inputs(D, X, A, W, b, n_cores=N_CORES, mode=EXACT):
    """Build per-core input maps (pure slicing / dtype re-encoding)."""
    exact = (mode == 'exact') or (mode is True)
    FDT = np.float16 if mode == 'fp16' else BF16
    n, f_in = X.shape
    f_out = W.shape[0]
    rows = n // n_cores
    C = n // 128
    nb = n // 128

    # A is 0/1: cast to 16-bit float is exact
    if mode == 'fp16':
        A_bf = np.ascontiguousarray(A).astype(np.float16).view(np.uint16)
    else:
        A_bf = (np.ascontiguousarray(A).view(np.uint32) >> 16).astype(np.uint16)
    dvec = np.ascontiguousarray(np.diagonal(D)).astype(np.float32)
    w_t = np.ascontiguousarray(W.T).astype(np.float32)
    bias_row = np.broadcast_to(b.astype(np.float32), (128, f_out)).copy()

    n_jblk = n // 128
    p = np.arange(128)
    w2reg = np.zeros((128, n_jblk, C), dtype=FDT)
    vals = (2.0 ** (100.0 - p)).astype(FDT)
    for bb in range(n_jblk):
        w2reg[p, bb, bb] = vals

    ident = np.eye(128, dtype=FDT)
    i2c227 = np.broadcast_to(
        (256 * np.arange(C) + 227).astype(np.int32), (128, C)
    ).copy()
    iq = np.broadcast_to((256.0 * np.arange(C)).astype(np.float32), (128, C)).copy()
    ir = np.broadcast_to(np.arange(128).astype(np.float32), (128, 128)).copy()

    shared = {
        "dvec": dvec,
        "w_t": w_t,
        "bias_row": bias_row,
        "w2reg": w2reg,
        "ident": ident,
        "i2c227": i2c227,
        "iota_q": iq,
        "iota_r": ir,
    }
    if exact:
        shared["x_f32"] = np.ascontiguousarray(X).astype(np.float32)
    else:
        shared["x_bf"] = np.ascontiguousarray(X).astype(FDT)

    in_maps = []
    for c in range(n_cores):
        m = dict(shared)
        m["a_sh"] = A_bf[c * rows:(c + 1) * rows, :].view(FDT)
        in_maps.append(m)
    return in_maps


def kernel(D, X, A, W, b):
    from concourse.bass_utils import run_bass_kernel_spmd

    n, f_in = X.shape
    f_out = W.shape[0]
    rows = n // N_CORES
    nc = _get_nc(rows, n, f_in, f_out, EXACT)
    in_maps = host_inputs(D, X, A, W, b, N_CORES, EXACT)
    res = run_bass_kernel_spmd(nc, in_maps, core_ids=list(range(N_CORES)))
    out = np.concatenate([r["out_sh"] for r in res.results], axis=0)
    return out.astype(np.float32)



# revision 3
# speedup vs baseline: 2.6978x; 2.6978x over previous
"""GCN-style message passing kernel for Trainium2 (8 NeuronCores).

Math (see reference):
    deg    = diag(D)                      (== row sums of A by construction)
    j0(i)  = argmax_j (A[i,j] > 0)        (first neighbor; self-loops ensure >=1)
    coeff  = A * outer(1/sqrt(deg[j0]), 1/sqrt(deg))
    out    = leaky_relu((coeff @ X) @ W.T + b, 0.01)

Decomposition per core (rows sharded, 1024 rows/core):
    agg   = diag(r0) @ A_sh @ (diag(r) @ X)       r = 1/sqrt(deg), r0 = 1/sqrt(deg[j0])
    out   = leaky_relu(agg @ W.T + b)

Single bf16 pass (A is 0/1 so it is exact in bf16; X*r rounds to bf16 once,
~3e-3 worst-case output error vs the 2e-2 gate). A^T is pre-transposed on the
host so every device DMA is a contiguous 2KB-per-partition-line slab load.
Xs = diag(r) @ X is pre-scaled on the host and shipped interleaved with the
"position" matrix W2 (w[j] = 2^(100-j%128), one column per 128-node chunk) as
one [128, n_jblk, 320] tile. deg[j0] is recovered on-device:
  - the fused matmul produces s[i,c] whose f32 EXPONENT encodes the first
    neighbor's offset within chunk c,
  - bit tricks + a free-dim min-reduce give first_j = 128*c* + jl*,
  - deg[first_j] is then gathered with a tiny bilinear form:
    onehot(c*)^T @ Dmat dotted with onehot(jl*), Dmat[q,r] = deg[128q+r].
"""

import numpy as np
import ml_dtypes

BF16 = ml_dtypes.bfloat16

N_NODES = 8192
F_IN = 256
F_OUT = 256
N_CORES = 8
ROWS = N_NODES // N_CORES  # rows per core

_BUILT = {}


def _build_nc(rows, n_nodes, f_in, f_out):
    import concourse.bass as bass
    import concourse.tile as tile
    from concourse import bacc, mybir

    f32 = mybir.dt.float32
    bf = mybir.dt.bfloat16
    i32 = mybir.dt.int32
    Alu = mybir.AluOpType

    n_jblk = n_nodes // 128     # contraction blocks
    n_iblk = rows // 128        # output row blocks per core
    C = n_jblk                  # 128-node chunks (s columns)
    NB = n_jblk
    W_COLS = f_in + C           # fused moving operand width
    assert C <= 128 and n_nodes % 128 == 0 and rows % 128 == 0
    assert f_in % 128 == 0 and f_out <= 512

    nc = bacc.Bacc("TRN2", target_bir_lowering=False, debug=False)
    # A^T shard: [n_nodes, rows] so slab loads are contiguous (no DMA transpose)
    a_t = nc.dram_tensor("a_t", [n_nodes, rows], bf, kind="ExternalInput")
    dvec = nc.dram_tensor("dvec", [n_nodes], f32, kind="ExternalInput")
    # [Xs | W2] interleaved per j-block, host-prescaled by r = 1/sqrt(deg)
    xsw_d = nc.dram_tensor("xsw", [128, n_jblk, W_COLS], bf, kind="ExternalInput")
    wthi_d = nc.dram_tensor("wthi", [128, f_in // 128, f_out], bf, kind="ExternalInput")
    wtlo_d = nc.dram_tensor("wtlo", [128, f_in // 128, f_out], bf, kind="ExternalInput")
    bias_row = nc.dram_tensor("bias_row", [128, f_out], f32, kind="ExternalInput")
    ident_d = nc.dram_tensor("ident", [128, 128], bf, kind="ExternalInput")
    i2c227_d = nc.dram_tensor("i2c227", [128, C], i32, kind="ExternalInput")
    iq_d = nc.dram_tensor("iota_q", [128, C], f32, kind="ExternalInput")
    ir_d = nc.dram_tensor("iota_r", [128, 128], f32, kind="ExternalInput")
    out_d = nc.dram_tensor("out_sh", [rows, f_out], f32, kind="ExternalOutput")

    nfi = f_in // 128  # fi blocks for second matmul

    with tile.TileContext(nc) as tc:
        with (
            tc.tile_pool(name="singles", bufs=1) as singles,
            tc.tile_pool(name="apool", bufs=4) as apool,
            tc.tile_pool(name="work", bufs=2) as work,
            tc.tile_pool(name="pspool", bufs=8, space="PSUM") as pspool,
        ):
            # ---- constants / prep ----
            ident = singles.tile([128, 128], bf)
            nc.gpsimd.dma_start(ident[:], ident_d[:])
            i2c227 = singles.tile([128, C], i32)
            nc.gpsimd.dma_start(i2c227[:], i2c227_d[:])
            iq = singles.tile([128, C], f32)
            nc.gpsimd.dma_start(iq[:], iq_d[:])
            ir = singles.tile([128, 128], f32)
            nc.gpsimd.dma_start(ir[:], ir_d[:])
            bias_t = singles.tile([128, f_out], f32)
            nc.gpsimd.dma_start(bias_t[:], bias_row[:])

            # degree lookup: Dmat[q, r] = deg[128q + r]
            dmat_f = singles.tile([C, 128], f32)
            nc.gpsimd.dma_start(dmat_f[:], dvec[:].rearrange("(q r) -> q r", r=128))
            dmat_b = singles.tile([C, 128], bf)
            nc.vector.tensor_copy(dmat_b[:], dmat_f[:])

            wthi = singles.tile([128, nfi, f_out], bf)
            nc.gpsimd.dma_start(wthi[:], wthi_d[:])
            wtlo = singles.tile([128, nfi, f_out], bf)
            nc.gpsimd.dma_start(wtlo[:], wtlo_d[:])

            # moving operand: [Xs | W2] per j-block, one shot
            xsw = singles.tile([128, n_jblk, W_COLS], bf)
            nc.sync.dma_start(xsw[:], xsw_d[:])

            # ---- main accumulation: agg = A_sh @ Xs ; s = A_sh @ W2
            ps_main = [
                pspool.tile([128, W_COLS], f32, tag="ps", name=f"ps_main{i}")
                for i in range(n_iblk)
            ]
            for jb in range(n_jblk):
                aslab = apool.tile([128, rows], bf, tag="aslab")
                nc.sync.dma_start(aslab[:], a_t[jb * 128:(jb + 1) * 128, :])
                for ib in range(n_iblk):
                    nc.tensor.matmul(
                        ps_main[ib][:, 0:W_COLS],
                        aslab[:, ib * 128:(ib + 1) * 128],
                        xsw[:, jb, :],
                        start=(jb == 0),
                        stop=(jb == n_jblk - 1),
                    )

            # ---- per row-block epilogue ----
            for ib in range(n_iblk):
                # drain psum: s and unscaled agg -> SBUF (releases the bank)
                s_sb = work.tile([128, C], f32, tag="s_sb")
                nc.scalar.copy(s_sb[:], ps_main[ib][:, f_in:W_COLS])
                agg_raw = work.tile([128, f_in], f32, tag="agg_raw")
                nc.scalar.copy(agg_raw[:], ps_main[ib][:, 0:f_in])

                e_u = work.tile([128, C], i32, tag="e_u")
                nc.vector.tensor_scalar(
                    e_u[:], s_sb[:].bitcast(i32), 23, None,
                    op0=Alu.logical_shift_right,
                )
                key = work.tile([128, C], i32, tag="key")
                nc.vector.scalar_tensor_tensor(
                    key[:], e_u[:], -1, i2c227[:], op0=Alu.mult, op1=Alu.add
                )
                msk = work.tile([128, C], i32, tag="msk")
                nc.vector.tensor_scalar(
                    msk[:], e_u[:], 0, 1 << 20, op0=Alu.is_equal, op1=Alu.mult
                )
                key2 = work.tile([128, C], i32, tag="key2")
                nc.vector.tensor_tensor(key2[:], key[:], msk[:], Alu.add)
                kmin = work.tile([128, 1], i32, tag="kmin")
                nc.vector.tensor_reduce(
                    kmin[:], key2[:], axis=mybir.AxisListType.X, op=Alu.min
                )
                # kmin = 256*c + jl  (c = chunk, jl = offset in chunk)
                jl2_i = work.tile([128, 1], i32, tag="jl2_i")
                nc.vector.tensor_scalar(
                    jl2_i[:], kmin[:], 127, None, op0=Alu.bitwise_and
                )
                c128_i = work.tile([128, 1], i32, tag="c128_i")
                nc.vector.tensor_scalar(
                    c128_i[:], kmin[:], -256, None, op0=Alu.bitwise_and
                )
                jl2_f = work.tile([128, 1], f32, tag="jl2_f")
                nc.vector.tensor_copy(jl2_f[:], jl2_i[:])
                c128_f = work.tile([128, 1], f32, tag="c128_f")
                nc.vector.tensor_copy(c128_f[:], c128_i[:])

                # onehots; gather deg[first_j] via oq^T @ Dmat then dot with or
                oq = work.tile([128, C], bf, tag="oq")
                nc.vector.tensor_scalar(
                    oq[:], iq[:], c128_f[:], None, op0=Alu.is_equal
                )
                orf = work.tile([128, 128], f32, tag="orf")
                nc.vector.tensor_scalar(
                    orf[:], ir[:], jl2_f[:], None, op0=Alu.is_equal
                )
                p_oqT = pspool.tile([C, 128], bf, tag="ps")
                nc.tensor.transpose(p_oqT[:], oq[:], ident[:])
                oqT = work.tile([C, 128], bf, tag="oqT")
                nc.scalar.copy(oqT[:], p_oqT[:])
                t1 = pspool.tile([128, 128], f32, tag="ps")
                nc.tensor.matmul(t1[:], oqT[:], dmat_b[:], start=True, stop=True)
                t1s = work.tile([128, 128], f32, tag="t1s")
                nc.scalar.copy(t1s[:], t1[:])
                ttr_scr = work.tile([128, 128], f32, tag="ttr_scr")
                nc.vector.tensor_tensor(ttr_scr[:], t1s[:], orf[:], Alu.mult)
                dj0 = work.tile([128, 1], f32, tag="dj0")
                nc.vector.reduce_sum(
                    dj0[:], ttr_scr[:], axis=mybir.AxisListType.X
                )
                sq0 = work.tile([128, 1], f32, tag="sq0")
                nc.scalar.sqrt(sq0[:], dj0[:])
                r0 = work.tile([128, 1], f32, tag="r0")
                nc.vector.reciprocal(r0[:], sq0[:])

                # agg scaled by r0, cast, transpose for the W matmul
                agg_b = work.tile([128, f_in], bf, tag="agg_b")
                nc.vector.tensor_scalar_mul(agg_b[:], agg_raw[:], r0[:])

                aggTs = []
                for h in range(nfi):
                    p_aT = pspool.tile([128, 128], bf, tag="ps")
                    nc.tensor.transpose(
                        p_aT[:], agg_b[:, h * 128:(h + 1) * 128], ident[:]
                    )
                    aT = work.tile([128, 128], bf, tag=f"aT{h}")
                    nc.scalar.copy(aT[:], p_aT[:])
                    aggTs.append(aT)

                ps2 = pspool.tile([128, f_out], f32, tag="ps")
                prods = []
                for h in range(nfi):
                    prods.append((aggTs[h], wthi[:, h, :]))
                    prods.append((aggTs[h], wtlo[:, h, :]))
                for pi, (lhs, rhs) in enumerate(prods):
                    nc.tensor.matmul(
                        ps2[:], lhs[:], rhs,
                        start=(pi == 0), stop=(pi == len(prods) - 1),
                    )

                z = work.tile([128, f_out], f32, tag="z")
                nc.vector.tensor_add(z[:], ps2[:], bias_t[:])
                out_t = work.tile([128, f_out], f32, tag="out_t")
                nc.vector.scalar_tensor_tensor(
                    out_t[:], z[:], 0.01, z[:], op0=Alu.mult, op1=Alu.max
                )
                nc.sync.dma_start(
                    out_d[ib * 128:(ib + 1) * 128, :], out_t[:]
                )

    nc.finalize()
    return nc


def _get_nc(rows, n_nodes, f_in, f_out):
    key = (rows, n_nodes, f_in, f_out)
    if key not in _BUILT:
        _BUILT[key] = _build_nc(*key)
    return _BUILT[key]


def host_inputs(D, X, A, W, b, n_cores=N_CORES):
    """Build per-core input maps (pure slicing / dtype re-encoding)."""
    n, f_in = X.shape
    f_out = W.shape[0]
    rows = n // n_cores
    C = n // 128
    n_jblk = n // 128

    # A is 0/1: truncation to bf16 is exact. Pre-transpose so the device
    # reads contiguous [128, rows] slabs.
    A_bf = (np.ascontiguousarray(A).view(np.uint32) >> 16).astype(np.uint16)
    dvec = np.ascontiguousarray(np.diagonal(D)).astype(np.float32)
    r = 1.0 / np.sqrt(dvec)

    # [Xs | W2] moving operand, host-prescaled and pre-laid-out
    Xs = (np.ascontiguousarray(X).astype(np.float32) * r[:, None]).astype(BF16)
    p = np.arange(128)
    xsw = np.zeros((128, n_jblk, f_in + C), dtype=BF16)
    xsw[:, :, 0:f_in] = Xs.reshape(n_jblk, 128, f_in).transpose(1, 0, 2)
    vals = (2.0 ** (100.0 - p)).astype(BF16)
    for bb in range(n_jblk):
        xsw[p, bb, f_in + bb] = vals

    w_t = np.ascontiguousarray(W.T).astype(np.float32)  # [f_in, f_out]
    nfi = f_in // 128
    wt_r = w_t.reshape(nfi, 128, f_out).transpose(1, 0, 2)  # [128, nfi, f_out]
    wthi = wt_r.astype(BF16)
    wtlo = (wt_r - wthi.astype(np.float32)).astype(BF16)

    bias_row = np.broadcast_to(b.astype(np.float32), (128, f_out)).copy()
    ident = np.eye(128, dtype=BF16)
    i2c227 = np.broadcast_to(
        (256 * np.arange(C) + 227).astype(np.int32), (128, C)
    ).copy()
    iq = np.broadcast_to((256.0 * np.arange(C)).astype(np.float32), (128, C)).copy()
    ir = np.broadcast_to(np.arange(128).astype(np.float32), (128, 128)).copy()

    shared = {
        "dvec": dvec,
        "xsw": xsw,
        "wthi": wthi,
        "wtlo": wtlo,
        "bias_row": bias_row,
        "ident": ident,
        "i2c227": i2c227,
        "iota_q": iq,
        "iota_r": ir,
    }

    in_maps = []
    for c in range(n_cores):
        m = dict(shared)
        m["a_t"] = np.ascontiguousarray(
            A_bf[c * rows:(c + 1) * rows, :].T
        ).view(BF16)
        in_maps.append(m)
    return in_maps


def kernel(D, X, A, W, b):
    from concourse.bass_utils import run_bass_kernel_spmd

    n, f_in = X.shape
    f_out = W.shape[0]
    rows = n // N_CORES
    nc = _get_nc(rows, n, f_in, f_out)
    in_maps = host_inputs(D, X, A, W, b, N_CORES)
    res = run_bass_kernel_spmd(nc, in_maps, core_ids=list(range(N_CORES)))
    out = np.concatenate([r["out_sh"] for r in res.results], axis=0)
    return out.astype(np.float32)
